# revision 1
# baseline (speedup 1.0000x reference)
"""DifferentiableSoftKMeans kernel — data-parallel over batch on 8 NeuronCores.

Sharding: B=8 images, one per core (jax.pmap over devices). Cross-core
communication per the sharding hint: per-BN-layer channel statistics
(mean/var over B,H,W) and the per-iteration [K,2]/[K] spatial-center sums
are all-reduced with lax.pmean; feature-center updates are per-batch (bmm)
and stay fully local. Output is the argmax cluster map, int32 [8,256,256].
"""

import numpy as np
from functools import partial

import jax
import jax.numpy as jnp
from jax import lax

N_CLUSTERS = 16
HIDDEN = 32
TEMPERATURE = 1.0
SPATIAL_WEIGHT = 10.0
N_ITERATIONS = 3
BN_EPS = 1e-5

N_CORES = 8
B, C, H, W = 8, 64, 256, 256


def _init_spatial_centers(k):
    gh = int(k ** 0.5)
    gw = (k + gh - 1) // gh
    c = []
    for i in range(gh):
        for j in range(gw):
            if len(c) < k:
                y = (i + 0.5) / gh * 2 - 1
                x = (j + 0.5) / gw * 2 - 1
                c.append([x, y])
    return jnp.asarray(np.array(c, dtype=np.float32))


def _conv(x, w, b, pad):
    y = lax.conv_general_dilated(x, w, (1, 1), [(pad, pad), (pad, pad)],
                                 dimension_numbers=('NCHW', 'OIHW', 'NCHW'))
    return y + b[None, :, None, None]


def _bn_dist(x, gamma, beta):
    # training-mode batch norm with global (cross-core) batch statistics,
    # two-pass mean/var to match single-device numerics closely
    m = lax.pmean(x.mean(axis=(0, 2, 3)), "b")
    d = x - m[None, :, None, None]
    v = lax.pmean((d * d).mean(axis=(0, 2, 3)), "b")
    xn = d * lax.rsqrt(v[None, :, None, None] + BN_EPS)
    return xn * gamma[None, :, None, None] + beta[None, :, None, None]


def _prelu(x, a):
    return jnp.where(x >= 0, x, a * x)


def _cdist(a, b):
    a2 = jnp.sum(a * a, axis=-1, keepdims=True)
    b2 = jnp.sum(b * b, axis=-1)[..., None, :]
    ab = jnp.matmul(a, jnp.swapaxes(b, -1, -2))
    d2 = jnp.maximum(a2 + b2 - 2.0 * ab, 1e-12)
    return jnp.sqrt(d2)


@partial(jax.pmap, axis_name="b",
         in_axes=(0,) + (None,) * 14,
         static_broadcasted_argnums=())
def _run_core(x, w1, b1, g1, be1, a1, w2, b2, g2, be2, a2, w3, b3,
              feature_centers, spatial):
    # x: [1, C, H, W] local shard
    f = _prelu(_bn_dist(_conv(x, w1, b1, 1), g1, be1), a1)
    f = _prelu(_bn_dist(_conv(f, w2, b2, 1), g2, be2), a2)
    f = _conv(f, w3, b3, 0)
    N = H * W
    feats = f.transpose(0, 2, 3, 1).reshape(1, N, HIDDEN)

    fc = jnp.broadcast_to(feature_centers[None], (1, N_CLUSTERS, HIDDEN))
    sc = _init_spatial_centers(N_CLUSTERS)
    soft = None
    for _ in range(N_ITERATIONS):
        fd = _cdist(feats, fc)                      # [1,N,K]
        sd = _cdist(spatial, sc)                    # [N,K]
        total = fd + SPATIAL_WEIGHT * sd[None]
        soft = jax.nn.softmax(-total / TEMPERATURE, axis=2)
        wsum = soft.sum(axis=1)[:, :, None]         # [1,K,1]
        fc = jnp.einsum('bnk,bnd->bkd', soft, feats) / (wsum + 1e-6)
        # spatial-center update: ws = soft.mean(axis=0).T over the GLOBAL
        # batch -> all-reduce only the [K,2] weighted sums and [K] masses
        S = lax.pmean(jnp.einsum('nk,nd->kd', soft[0], spatial), "b")  # [K,2]
        m = lax.pmean(soft[0].sum(axis=0), "b")                        # [K]
        sc = S / (m[:, None] + 1e-6)
    hard = jnp.argmax(soft, axis=2).astype(jnp.int32)
    return hard.reshape(1, H, W)


_spatial_cache = None


def _spatial():
    global _spatial_cache
    if _spatial_cache is None:
        yy, xx = np.meshgrid(np.linspace(-1.0, 1.0, H, dtype=np.float64),
                             np.linspace(-1.0, 1.0, W, dtype=np.float64),
                             indexing='ij')
        _spatial_cache = np.stack([xx, yy], axis=-1).reshape(H * W, 2).astype(np.float32)
    return _spatial_cache


def kernel(x, w1, b1, g1, be1, a1, w2, b2, g2, be2, a2, w3, b3,
           feature_centers):
    x = np.asarray(x, dtype=np.float32).reshape(N_CORES, 1, C, H, W)
    args = [np.asarray(v, dtype=np.float32) for v in
            (w1, b1, g1, be1, a1, w2, b2, g2, be2, a2, w3, b3,
             feature_centers)]
    out = _run_core(x, *args, _spatial())        # [8, 1, H, W]
    return np.asarray(out).reshape(B, H, W).astype(np.int32)
